# revision 1
# baseline (speedup 1.0000x reference)
"""Multi-scale patch pooling (gather + segment-mean) for CLIP-AD on 8 trn2 cores.

The reference computes, per batch element b:
    large[b, g, :] = mean over l of tokens[b, large_mask[l, g], :]   (9-elt mean, 169 groups)
    mid[b, g, :]   = mean over l of tokens[b, mid_mask[l, g], :]     (4-elt mean, 196 groups)
    cls[b, 0, :]   = mean over t of tokens[b, t, :]                  (225-elt mean)
    out = concat([large, mid, cls], axis=1)                          # [B, 366, D]

Per batch this is exactly out_b = diag(s) @ W01 @ tokens_b where W01 is a tiny
[366, 225] 0/1 membership-count matrix built host-side from the masks (handles
arbitrary / duplicate indices) and s[g] = 1/group_size. The device kernel runs
the matmul on the tensor engine in float32r (full-rate single-pass fp32 mode,
~1e-4 rel err); integer W01 entries are exact. The per-group 1/L scale is
applied during the PSUM->SBUF evacuation copy on DVE/ACT.

DMA design (all bulk traffic on gpsimd/SWDGE):
  * SWDGE picks a transfer's SDMA engines from its write-side address
    (~14 KB granularity on SBUF, ~655 KB on DRAM), so: token slots live in
    single-buffer pools interleaved across the SBUF address space (concurrent
    loads drain on distinct engines), and stores rotate engines naturally via
    their DRAM addresses.
  * Loads are software-pipelined LOOK pairs ahead; stores are emitted one
    pair late so their waits are satisfied on arrival — the gpsimd queue is
    in-order and a waiting store would block every load queued behind it.
  * One load per batch pair (tokens repacked host-side to [bp/2, 113, 4D])
    and one store per pair (o tile holds both batches; W's columns are
    permuted so output group 3p+mi lands on PSUM partition p of m-tile mi,
    making out[2j:2j+2] partition-contiguous).

Sharding: pure data parallel on batch — 64 batches per core; weights replicated.
"""

import numpy as np

B, T, D = 512, 225, 896
GL, LL = 169, 9
GM, LM = 196, 4
G = GL + GM + 1  # 366
N_CORES = 8
BP = B // N_CORES  # 64

KP = 113                      # k-chunk partition count (225 -> 113 + 112)
MP = G // 3                   # 122 partitions per m-tile (groups strided by 3)
_K_TILES = ((0, 113), (113, 112))
_N_TILES = ((0, 512), (512, 384))
ROWPAD = 16                   # f32 elems of pad per packed row (64 B)

_CACHE = {}


def _get_nc(bp=BP):
    if bp in _CACHE:
        return _CACHE[bp]
    from contextlib import ExitStack

    import concourse.bacc as bacc
    import concourse.mybir as mybir
    import concourse.tile as tile

    f32 = mybir.dt.float32
    f32r = mybir.dt.float32r

    nc = bacc.Bacc("TRN2", target_bir_lowering=False, debug=False)
    # tokens4[j, p, :] = concat over (bb in 0..1, c in 0..1) of
    #   row(2j+bb, c*113+p) (896 f32 each), plus 64 B pad.
    tokens4 = nc.dram_tensor(
        "tokens4", [bp // 2, KP, 4 * D + ROWPAD], f32r, kind="ExternalInput"
    ).ap()
    # w01T[t, mi*122 + p] = W01[3p + mi, t]
    w01T = nc.dram_tensor("w01T", [T, G], f32r, kind="ExternalInput").ap()
    # scale[mi*122 + p] = 1/L of group 3p+mi
    scale = nc.dram_tensor("scale", [G, 1], f32, kind="ExternalInput").ap()
    out = nc.dram_tensor("out", [bp, G, D], f32, kind="ExternalOutput").ap()

    NTOK = 7  # token slots (one in-flight load each, distinct engines)
    NOB = 4   # per-pair output slots

    with tile.TileContext(nc) as tc:
        with ExitStack() as ctx:
            tok_pools = []
            o_pools = []
            for s in range(NTOK):
                tok_pools.append(
                    ctx.enter_context(tc.tile_pool(name=f"tokp{s}", bufs=1))
                )
                if s < NOB:
                    o_pools.append(
                        ctx.enter_context(tc.tile_pool(name=f"obp{s}", bufs=1))
                    )
            wpool = ctx.enter_context(tc.tile_pool(name="w", bufs=1))
            pspool = ctx.enter_context(
                tc.tile_pool(name="ps", bufs=8, space="PSUM")
            )

            # Warm-up ops: the first ACT/DVE instructions pick up extra
            # table-load waits in lowering; give them dummies with no
            # cross-engine deps so real ops keep their wait budget.
            warm = wpool.tile([128, 1], f32, tag="warm")
            nc.gpsimd.memset(warm[:], 0.0)
            nc.scalar.activation(
                warm[:], warm[:], mybir.ActivationFunctionType.Copy
            )
            nc.vector.tensor_copy(warm[:], warm[:])

            w_sb = []
            for ki, (k0, ksz) in enumerate(_K_TILES):
                wt = wpool.tile([128, G], f32r, tag=f"w{ki}")
                nc.gpsimd.dma_start(wt[:ksz, :], w01T[k0 : k0 + ksz, :])
                w_sb.append(wt)
            sc_sb = []
            for mi in range(3):
                st = wpool.tile([128, 1], f32, tag=f"sc{mi}")
                nc.gpsimd.dma_start(st[:MP, :], scale[mi * MP : (mi + 1) * MP, :])
                sc_sb.append(st)

            LOOK = 4
            npair = bp // 2
            tks = {}

            def emit_load(j):
                tk = tok_pools[j % NTOK].tile(
                    [128, 4 * D], f32r, name="tok", tag="tok"
                )
                nc.gpsimd.dma_start(tk[:KP, :], tokens4[j, :, : 4 * D])
                tks[j] = tk

            pending_stores = []

            def flush_stores():
                for dst, src in pending_stores:
                    nc.gpsimd.dma_start(dst, src)
                pending_stores.clear()

            cp = 0
            for j in range(LOOK):
                emit_load(j)
            for j in range(npair):
                if j + LOOK < npair:
                    emit_load(j + LOOK)
                flush_stores()
                tk = tks.pop(j)
                o = o_pools[j % NOB].tile([128, 6 * D], f32, name="ob", tag="ob")
                for bb in range(2):
                    for mi in range(3):
                        pss = [
                            pspool.tile([128, 512], f32, name="ps", tag="ps")
                            for _ in _N_TILES
                        ]
                        for ki, (k0, ksz) in enumerate(_K_TILES):
                            base = bb * 2 * D + ki * D
                            for ni, (n0, nsz) in enumerate(_N_TILES):
                                nc.tensor.matmul(
                                    pss[ni][:MP, :nsz],
                                    w_sb[ki][:ksz, mi * MP : (mi + 1) * MP],
                                    tk[:ksz, base + n0 : base + n0 + nsz],
                                    start=(ki == 0),
                                    stop=(ki == 1),
                                )
                        # PSUM -> SBUF evacuation with the per-group 1/L
                        # scale. DMA cannot read PSUM; alternate DVE / ACT.
                        for ni, (n0, nsz) in enumerate(_N_TILES):
                            dst = o[
                                :MP,
                                bb * 3 * D + mi * D + n0 : bb * 3 * D
                                + mi * D
                                + n0
                                + nsz,
                            ]
                            if cp % 2 == 1:
                                nc.scalar.activation(
                                    dst,
                                    pss[ni][:MP, :nsz],
                                    mybir.ActivationFunctionType.Copy,
                                    scale=sc_sb[mi][:MP, :],
                                )
                            else:
                                nc.vector.tensor_scalar_mul(
                                    dst, pss[ni][:MP, :nsz], sc_sb[mi][:MP, :]
                                )
                            cp += 1
                # Store both batches of the pair: partition p holds groups
                # 3p..3p+2 of each batch -> out[2j:2j+2] is contiguous per
                # (partition, batch). Deferred one pair (see pending_stores).
                pending_stores.append(
                    (
                        out[2 * j : 2 * j + 2].rearrange(
                            "b (p c) d -> p b (c d)", c=3
                        ),
                        o[:MP, :].rearrange("p (b x) -> p b x", b=2),
                    )
                )
            flush_stores()

    nc.compile()
    _CACHE[bp] = nc
    return nc


def _host_prep(tokens_full, large_mask, mid_mask):
    """Pack tokens for paired loads; build 0/1 weight + scale tensors."""
    bsz = tokens_full.shape[0]

    # Pack: tokens4[j, p, (bb, c)] = row(2j+bb, c*113+p); row 225 of a batch
    # aliases the next batch's row 0 (junk, partition 112 of chunk 1 unused);
    # one zero pad row covers the very last access.
    flat = np.concatenate(
        [tokens_full.reshape(bsz * T, D), np.zeros((1, D), np.float32)], axis=0
    )
    jj = np.arange(bsz // 2)[:, None, None, None]
    pp = np.arange(KP)[None, :, None, None]
    bb = np.arange(2)[None, None, :, None]
    cc = np.arange(2)[None, None, None, :]
    idx = np.minimum((2 * jj + bb) * T + cc * KP + pp, bsz * T)
    tokens4 = flat[idx].reshape(bsz // 2, KP, 4 * D)
    # 64 B pad per row: keeps DMA source runs non-contiguous (SWDGE would
    # otherwise merge them and chunk the merged stream onto few engines).
    tokens4 = np.concatenate(
        [tokens4, np.zeros((bsz // 2, KP, ROWPAD), np.float32)], axis=2
    )

    W = np.zeros((G, T), np.float32)
    rows = np.arange(GL)
    for l in range(large_mask.shape[0]):
        np.add.at(W, (rows, large_mask[l]), 1.0)
    rows = GL + np.arange(GM)
    for l in range(mid_mask.shape[0]):
        np.add.at(W, (rows, mid_mask[l]), 1.0)
    W[G - 1, :] = 1.0

    s = np.empty(G, np.float32)
    s[:GL] = 1.0 / large_mask.shape[0]
    s[GL : GL + GM] = 1.0 / mid_mask.shape[0]
    s[G - 1] = 1.0 / T

    # Permute groups so m-tile mi, partition p <-> group 3p+mi.
    perm = np.concatenate([np.arange(mi, G, 3) for mi in range(3)])
    w01T = np.ascontiguousarray(W[perm].T)  # [T, G] f32
    s_perm = np.ascontiguousarray(s[perm].reshape(G, 1))
    return tokens4, w01T, s_perm


def _in_maps(tokens4, w01T, s, n_cores=N_CORES):
    jp = tokens4.shape[0] // n_cores
    return [
        {
            "tokens4": np.ascontiguousarray(tokens4[c * jp : (c + 1) * jp]),
            "w01T": w01T,
            "scale": s,
        }
        for c in range(n_cores)
    ]


def kernel(**inputs):
    from concourse import bass_utils

    tokens_full = np.ascontiguousarray(np.asarray(inputs["patch_tokens"], np.float32))
    large = np.asarray(inputs["large_mask"]).astype(np.int64)
    mid = np.asarray(inputs["mid_mask"]).astype(np.int64)
    tokens4, w01T, s = _host_prep(tokens_full, large, mid)

    nc = _get_nc()
    res = bass_utils.run_bass_kernel_spmd(
        nc, _in_maps(tokens4, w01T, s), core_ids=list(range(N_CORES))
    )
    return np.concatenate(
        [res.results[c]["out"] for c in range(N_CORES)], axis=0
    ).astype(np.float32)

